# revision 20
# baseline (speedup 1.0000x reference)
"""Trainium2 Bass kernel for MQA causal attention (nn_GeminiAttention).

Reference computation (fp32):
    q = x @ wq + bq            [B,S,H,DK]   (H=16 heads)
    k = x @ wk + bk            [B,S,DK]     (shared across heads, MQA)
    v = x @ wv + bv            [B,S,DK]
    scores = q k^T / sqrt(DK), causal mask, softmax over keys
    out = (attn @ v) @ wo + bo [B,S,D]

Sharding: 8 cores = 2 (batch) x 4 (head groups of 4 heads). K/V replicated
per head group. Each core produces a partial output (its head group's slice
of the attention output times its wo rows); the host sums the 4 partials
per batch and adds bo.

On-device layout is fully "transposed" so no transposes are ever needed:
    xT   [KD, PT, S]  (host-transposed input, bf16, contiguous k-tiles)
    QT   [256, S] = wq_g^T x^T   (grouped per head pair on 128 partitions)
    KT   [64, S]  = wk^T x^T     (duplicated into both partition halves so
                                  lhsT/rhs base partitions match per head)
    V65  [S, 65]  = [x wv | 1]   (ones column makes the attention matmul
                                  also produce the softmax denominator Z)
    scoresT tile [t=128, q=512] = KT_tile^T.T @ QT_slice (K=dk=64)
    expT = exp(scoresT / 8)  (no max-subtraction: q,k ~ N(0,1) so scores/8
                              stay well inside fp32 exp range)
    causal masking via gpsimd.affine_select on diagonal tiles; fully-masked
    tiles are skipped entirely.
    attnoutT+Z psum [65, 512] = V65^T @ expT accumulated over t tiles
    psum evacuated to SBUF immediately; 1/Z via DVE reciprocal_approx_fast;
    1/Z broadcast across partitions via a K=1 PE matmul
    out partial [S, D] = attnoutT_g^T.T @ wo_g (K=64 per head, 4 heads acc)

The normalize + output projection for q-block j is emitted AFTER the
scores/AV stream of q-block j+1 (software pipelining) so the in-order PE
queue never stalls on the DVE reciprocal chain.

All matmul operands are bf16 (psum accumulation stays fp32); host casts
inputs and sums the bf16 partial outputs in fp32.
"""

import sys

sys.path.insert(0, "/opt/trn_rl_repo")

import ml_dtypes
import numpy as np

import concourse.bass as bass  # noqa: F401  (engine classes referenced via nc)
import concourse.mybir as mybir
import concourse.tile as tile
from concourse import bacc, bass_utils
from concourse.masks import make_identity

B, S, D, H, DK = 2, 2048, 1024, 16, 64
NCORES, GROUPS = 8, 4
H_PER = H // GROUPS          # 4 heads per core
GD = H_PER * DK              # 256 group hidden size
PT = 128                     # partition tile
NQ = 512                     # q free-dim block (one PSUM bank fp32)
NT = S // PT                 # 16 t tiles
NQB = S // NQ                # 4 q blocks
KD = D // PT                 # 8 contraction tiles over D

F32 = mybir.dt.float32
BF16 = mybir.dt.bfloat16
DT = BF16
NPDT = ml_dtypes.bfloat16

SKIP, FULL, PARTIAL = 0, 1, 2


_ACT_TABLES_PATCHED = False


def _patch_act_tables():
    """Force Exp and Ln onto the one act table containing both, so the
    per-normalize table reloads (1.3us each) disappear. Only set contents
    are edited; list order/length (= act_func_set_id) is preserved."""
    global _ACT_TABLES_PATCHED
    if _ACT_TABLES_PATCHED:
        return
    _ACT_TABLES_PATCHED = True
    from concourse import hw_specs

    orig = hw_specs.get_activation_tables

    @__import__("functools").cache
    def patched(module_arch):
        tables = dict(orig(module_arch))
        exp = mybir.ActivationFunctionType.Exp
        ln = mybir.ActivationFunctionType.Ln
        both = {name for name, s in tables.items() if exp in s and ln in s}
        if both:
            for name in tables:
                if name not in both and (exp in tables[name] or ln in tables[name]):
                    tables[name] = tables[name] - {exp, ln}
        return tables

    hw_specs.get_activation_tables = patched
    bacc.get_activation_tables = patched


def build_program(cls, use_bias):
    _patch_act_tables()
    nc = bacc.Bacc(None, target_bir_lowering=False)

    xT_d = nc.dram_tensor("xT", [KD, PT, S], DT, kind="ExternalInput")
    wq_d = nc.dram_tensor("wq", [PT, KD, GD], DT, kind="ExternalInput")
    wkv_d = nc.dram_tensor("wkv", [PT, KD, 2 * DK], DT, kind="ExternalInput")
    wo_d = nc.dram_tensor("wo", [PT, GD // PT, D], DT, kind="ExternalInput")
    out_d = nc.dram_tensor("out", [S, D], DT, kind="ExternalOutput")
    if use_bias:
        bq_d = nc.dram_tensor("bq", [1, GD], DT, kind="ExternalInput")
        bk_d = nc.dram_tensor("bk", [1, DK], DT, kind="ExternalInput")
        bv_d = nc.dram_tensor("bv", [1, DK], DT, kind="ExternalInput")

    out_t = out_d.rearrange("(t p) n -> t p n", p=PT)

    Exp = mybir.ActivationFunctionType.Exp
    mult = mybir.AluOpType.mult
    is_ge = mybir.AluOpType.is_ge

    with tile.TileContext(nc) as tc:
        with (
            nc.allow_low_precision("bf16 matmul operands are rounded by design"),
            tc.tile_pool(name="persist", bufs=1) as pp,
            tc.tile_pool(name="work", bufs=2) as wp,
            tc.tile_pool(name="expp", bufs=4) as ep,
            tc.tile_pool(name="outp", bufs=2) as op_,
            tc.tile_pool(name="ps_s", bufs=3, space="PSUM") as ps_sp,
            tc.tile_pool(name="ps_o", bufs=1, space="PSUM") as ps_op,
        ):
            # ---- persistent SBUF tiles; xT spread across 4 DMA queues so the
            #      8 MiB.. 4 MiB (bf16) load saturates HBM, first tiles first ----
            engs = [nc.sync, nc.scalar, nc.gpsimd]
            xT_sb = []
            for k in range(KD):
                t = pp.tile([PT, S], DT, name=f"xT{k}", tag=f"xT{k}")
                xT_sb.append(t)
            # issue order per queue: early k-tiles first, then weights (wq
            # needed by the first matmul, wo only at the output projection)
            wq_sb = pp.tile([PT, KD, GD], DT, name="wq_sb", tag="wq_sb")
            wkv_sb = pp.tile([PT, KD, 2 * DK], DT, name="wkv_sb", tag="wkv_sb")
            wo_sb = []
            for i in range(GD // PT):
                t = pp.tile([PT, D], DT, name=f"wo{i}", tag=f"wo{i}")
                wo_sb.append(t)

            nc.sync.dma_start(wq_sb[:], wq_d[:])
            nc.gpsimd.dma_start(wkv_sb[:], wkv_d[:])
            # half-tile chunks, j-major-ish: the first-half chunks of every
            # k-tile arrive first so KV(j0)/QT(jp0) can start earlier
            n = 0
            for j2 in range(2):
                for k in range(KD):
                    engs[n % 3].dma_start(
                        xT_sb[k][:, j2 * S // 2 : (j2 + 1) * S // 2],
                        xT_d[k][:, j2 * S // 2 : (j2 + 1) * S // 2],
                    )
                    n += 1
            for i in range(GD // PT):
                nc.sync.dma_start(wo_sb[i][:], wo_d[:, i, :])

            ones_f32 = pp.tile([PT, DK], F32, name="ones_f32", tag="ones_f32")
            nc.any.memset(ones_f32[:], 1.0)
            ones_sb = pp.tile([PT, DK], DT, name="ones_sb", tag="ones_sb")
            nc.vector.tensor_copy(ones_sb[:], ones_f32[:])
            ident_f32 = pp.tile([PT, PT], F32, name="ident_f32", tag="ident_f32")
            make_identity(nc, ident_f32[:])
            ident = pp.tile([PT, PT], DT, name="ident", tag="ident")
            nc.vector.tensor_copy(ident[:], ident_f32[:])

            if use_bias:
                bq_sb = pp.tile([1, GD], DT, name="bq_sb", tag="bq_sb")
                nc.sync.dma_start(bq_sb[:], bq_d[:])
                bk_sb = pp.tile([1, DK], DT, name="bk_sb", tag="bk_sb")
                nc.sync.dma_start(bk_sb[:], bk_d[:])
                bv_sb = pp.tile([1, DK], DT, name="bv_sb", tag="bv_sb")
                nc.sync.dma_start(bv_sb[:], bv_d[:])
                xones_f32 = pp.tile([1, S], F32, name="xones_f32", tag="xones_f32")
                nc.any.memset(xones_f32[:], 1.0)
                xones = pp.tile([1, S], DT, name="xones", tag="xones")
                nc.vector.tensor_copy(xones[:], xones_f32[:])

            QT_sb = [
                pp.tile([PT, S], DT, name=f"QT{i}", tag=f"QT{i}") for i in range(2)
            ]
            KT2 = pp.tile([PT, S], DT, name="KT2", tag="KT2")
            VT_sb = pp.tile([PT, S], DT, name="VT_sb", tag="VT_sb")
            V65 = [
                pp.tile([PT, DK + 1], DT, name=f"V65_{t}", tag=f"V65_{t}")
                for t in range(NT)
            ]
            # attention outputs for head pairs: heads 2i and 2i+1 stacked on
            # partitions [0:64] and [64:128] so the output projection runs with
            # a full K=128 contraction
            aoT = [
                pp.tile([PT, S], DT, name=f"aoT{i}", tag=f"aoT{i}")
                for i in range(GD // PT)
            ]

            _build_compute(
                nc, cls, use_bias,
                xT_sb, wq_sb, wkv_sb, wo_sb, ones_sb, ones_f32, ident,
                (bq_sb, bk_sb, bv_sb, xones) if use_bias else None,
                QT_sb, KT2, VT_sb, V65, aoT,
                wp, ep, op_, ps_sp, ps_op,
                out_t, Exp, mult, is_ge,
            )

    nc.compile()
    return nc


def _build_compute(
    nc, cls, use_bias,
    xT_sb, wq_sb, wkv_sb, wo_sb, ones_sb, ones_f32, ident,
    bias_tiles,
    QT_sb, KT2, VT_sb, V65, aoT,
    wp, ep, op_, ps_sp, ps_op,
    out_t, Exp, mult, is_ge,
):
    if use_bias:
        bq_sb, bk_sb, bv_sb, xones = bias_tiles
    Ln = mybir.ActivationFunctionType.Ln

    # ---- projection emitters (QT halves / KV blocks / V65 tiles) ----
    def emit_qt(jp):
        for m in range(GD // PT):
            psq = ps_sp.tile([PT, 2, NQ], F32, name="psq", tag="ps_s")
            for jj in range(2):
                j = jp * 2 + jj
                for k in range(KD):
                    nc.tensor.matmul(
                        psq[:, jj, :],
                        wq_sb[:, k, m * PT : (m + 1) * PT],
                        xT_sb[k][:, j * NQ : (j + 1) * NQ],
                        start=(k == 0),
                        stop=(k == KD - 1) and not use_bias,
                    )
                if use_bias:
                    nc.tensor.matmul(
                        psq[:, jj, :],
                        bq_sb[:, m * PT : (m + 1) * PT],
                        xones[:, j * NQ : (j + 1) * NQ],
                        start=False,
                        stop=True,
                    )
            nc.vector.tensor_copy(
                QT_sb[m][:, jp * 2 * NQ : (jp + 1) * 2 * NQ], psq[:]
            )

    def emit_kv(j):
        # fused K/V: psum rows [0:64] = KT block, [64:128] = VT block
        pskv = ps_sp.tile([PT, 2, NQ], F32, name="pskv", tag="ps_s")
        for k in range(KD):
            nc.tensor.matmul(
                pskv[:, 0, :],
                wkv_sb[:, k, :],
                xT_sb[k][:, j * NQ : (j + 1) * NQ],
                start=(k == 0),
                stop=(k == KD - 1) and not use_bias,
            )
        if use_bias:
            nc.tensor.matmul(
                pskv[0:DK, 0, :],
                bk_sb[:],
                xones[:, j * NQ : (j + 1) * NQ],
                start=False,
                stop=False,
            )
            nc.tensor.matmul(
                pskv[DK : 2 * DK, 0, :],
                bv_sb[:],
                xones[:, j * NQ : (j + 1) * NQ],
                start=False,
                stop=True,
                tile_position=(0, DK),
            )
        nc.vector.tensor_copy(KT2[0:DK, j * NQ : (j + 1) * NQ], pskv[0:DK, 0, :])
        nc.vector.tensor_copy(
            VT_sb[DK : 2 * DK, j * NQ : (j + 1) * NQ], pskv[DK : 2 * DK, 0, :]
        )
        nc.gpsimd.dma_start(
            KT2[DK : 2 * DK, j * NQ : (j + 1) * NQ],
            KT2[0:DK, j * NQ : (j + 1) * NQ],
        )

    def emit_v65(ts):
        # V65 = [VT^T | 1] via PE transpose
        for t in ts:
            ps_t = ps_sp.tile([PT, 2, NQ], DT, name="ps_t", tag="ps_s")
            nc.tensor.transpose(
                ps_t[:, 0, 0:DK],
                VT_sb[DK : 2 * DK, t * PT : (t + 1) * PT],
                ident[DK : 2 * DK, DK : 2 * DK],
            )
            nc.vector.tensor_copy(V65[t][:, 0:DK], ps_t[:, 0, 0:DK])
            nc.vector.tensor_copy(V65[t][:, DK : DK + 1], ones_f32[:, 0:1])

    # ---- attention: software-pipelined across q-blocks.
    #
    # Per stage qj the PE stream is [scores+AV i=0 | mid | scores+AV i=1];
    # `mid` carries the previous block's normalize (psb broadcasts inline in
    # the stream, DVE multiplies run under the i=1 scores) — for qj=0 it
    # carries the remaining projections instead. The output projection of
    # qj-1 follows the stream. 1/Z for a whole stage is ONE batched DVE
    # reciprocal over the 4 Z rows gathered to partitions {0,32,64,96}
    # (gpsimd DMAs), keeping both the ACT queue (exp-saturated) and the PE
    # queue free of it.

    def scores_av(qj, mid=None, mid2=None, last=False):
        a65s = []
        zr = wp.tile([PT, NQ], F32, name="zr", tag="zr")
        recbS = wp.tile([PT, NQ], DT, name="recbS", tag="recbS")
        recbS2 = wp.tile([1, NQ], DT, name="recbS2", tag="recbS2")
        for i in range(GD // PT):
            if i == 1 and mid is not None:
                mid()
            tis = [t for t in range(NT) if cls[t][qj] != SKIP]
            pso = [
                ps_op.tile([PT, NQ], F32, name=f"pso{hh}", tag=f"ps_o{hh}",
                           bufs=1)
                for hh in range(2)
            ]

            def emit_av(av):
                # AV runs one tile behind scores so the in-order PE queue
                # never waits on the exp of the tile it just produced
                expt, ti, colbase, wN, idx = av
                for hh in range(2):
                    nc.tensor.matmul(
                        pso[hh][0 : DK + 1, colbase:NQ],
                        V65[ti][:],
                        expt[:, hh, 0:wN],
                        start=(idx == 0),
                        stop=(idx == len(tis) - 1),
                    )

            pend = []
            for idx, ti in enumerate(tis):
                if i == 1 and idx == 2 and mid2 is not None:
                    mid2()
                partial = cls[ti][qj] == PARTIAL
                colbase = (ti - 4 * qj) * PT if partial else 0
                wN = NQ - colbase
                pss = ps_sp.tile([PT, 2, NQ], F32, name="pss", tag="ps_s")
                for hh, off in ((0, 0), (1, DK)):
                    nc.tensor.matmul(
                        pss[:, hh, 0:wN],
                        KT2[off : off + DK, ti * PT : (ti + 1) * PT],
                        QT_sb[i][
                            off : off + DK,
                            qj * NQ + colbase : (qj + 1) * NQ,
                        ],
                        start=True,
                        stop=True,
                    )
                expt = ep.tile([PT, 2, NQ], DT, name="expt", tag="expt")
                nc.scalar.activation(
                    expt[:, :, 0:wN], pss[:, :, 0:wN], Exp, scale=0.125
                )
                if partial:
                    # local cols [0:128) hold the diagonal; keep j >= p
                    # (one op covers both heads: affine coeff 0 on dim hh)
                    nc.gpsimd.affine_select(
                        expt[:, :, 0:PT],
                        expt[:, :, 0:PT],
                        pattern=[[0, 2], [1, PT]],
                        compare_op=is_ge,
                        fill=0.0,
                        base=0,
                        channel_multiplier=-1,
                    )
                pend.append((expt, ti, colbase, wN, idx))
                if len(pend) > 2:
                    emit_av(pend.pop(0))
            for av in pend:
                emit_av(av)

            for hh in range(2):
                # evacuate [AV | Z] rows immediately (psum bank frees) and
                # gather the Z row onto partition 32*(2i+hh) of zr
                a65 = wp.tile([PT, NQ], F32, name=f"a65_{i}{hh}",
                              tag=f"a65_{i}{hh}")
                nc.vector.tensor_copy(a65[0 : DK + 1, :], pso[hh][0 : DK + 1, :])
                p = 2 * i + hh
                nc.sync.dma_start(zr[p : p + 1, :], a65[DK : DK + 1, :])
                a65s.append(a65)
        # ONE batched reciprocal over the 4 gathered Z rows (DVE cost is
        # free-size only), then spread to matmul-legal base partitions
        zrec = wp.tile([PT, NQ], F32, name="zrec", tag="zrec")
        nc.vector.reciprocal(zrec[0:4, :], zr[0:4, :])
        recb4 = wp.tile([4, NQ], DT, name="recb4", tag="recb4")
        nc.vector.tensor_copy(recb4[:], zrec[0:4, :])
        for j, p in enumerate((0, 32, 64)):
            nc.sync.dma_start(recbS[p : p + 1, :], recb4[j : j + 1, :])
        nc.sync.dma_start(recbS2[0:1, :], recb4[3:4, :])
        return (a65s, recbS, recbS2)

    def normalize(qj, att):
        """broadcast 1/Z (PE, inline in the next score stream, reusing the
        freed AV psum bufs), normalize into aoT (DVE/gpsimd)."""
        a65s, recbS, recbS2 = att
        n = 0
        for i in range(GD // PT):
            for hh in range(2):
                a65 = a65s[n]
                n += 1
                idx = 2 * i + hh
                if idx < 3:
                    rsrc, p = recbS, 32 * idx
                else:
                    rsrc, p = recbS2, 0
                psb = ps_op.tile([PT, NQ], F32, name="psb", tag=f"ps_o{hh}",
                                 bufs=1)
                nc.tensor.matmul(
                    psb[0:DK, :],
                    ones_sb[p : p + 1, 0:DK],
                    rsrc[p : p + 1, :],
                    start=True,
                    stop=True,
                )
                if hh == 0:
                    nc.vector.tensor_tensor(
                        aoT[i][0:DK, qj * NQ : (qj + 1) * NQ],
                        a65[0:DK, :],
                        psb[0:DK, :],
                        mult,
                    )
                else:
                    # odd heads land on partitions [64:128] of the pair tile
                    # via a partition-shifting SBUF->SBUF DMA on the gpsimd
                    # queue (engines cannot cross partitions)
                    attn = wp.tile([DK, NQ], DT, name="attn", tag="attn")
                    nc.vector.tensor_tensor(
                        attn[:], a65[0:DK, :], psb[0:DK, :], mult
                    )
                    nc.gpsimd.dma_start(
                        aoT[i][DK : 2 * DK, qj * NQ : (qj + 1) * NQ], attn[:]
                    )

    def outproj(qj):
        for mq in range(4 * qj, 4 * qj + 4):
            osb = op_.tile([PT, 2, NQ], DT, name="osb", tag="osb", bufs=3)
            psf = ps_sp.tile([PT, 2, NQ], F32, name="psf", tag="ps_s")
            for nd in range(D // NQ):
                for i in range(GD // PT):
                    nc.tensor.matmul(
                        psf[:, nd, :],
                        aoT[i][:, mq * PT : (mq + 1) * PT],
                        wo_sb[i][:, nd * NQ : (nd + 1) * NQ],
                        start=(i == 0),
                        stop=(i == GD // PT - 1),
                    )
            nc.vector.tensor_copy(osb[:], psf[:])
            eng = nc.sync if mq % 2 == 0 else nc.scalar
            eng.dma_start(out_t[mq], osb[:, :, :].rearrange("p a b -> p (a b)"))

    # minimal projections for q-block 0, rest folded into its score stream
    # (KV first: its KT2-dup DMA + V65 chain hides under the QT matmuls)
    emit_kv(0)
    emit_qt(0)
    emit_v65(range(4))

    def mid0():
        emit_qt(1)
        for j in range(1, NQB):
            emit_kv(j)
        emit_v65(range(4, NT))

    prev = None
    for qj in range(NQB):
        if qj == 0:
            att = scores_av(0, mid=mid0)
        else:
            p_att, p_qj = prev
            att = scores_av(qj, mid=lambda: normalize(p_qj, p_att),
                            mid2=lambda: outproj(p_qj))
        prev = (att, qj)
    normalize(NQB - 1, prev[0])
    outproj(NQB - 1)


def _classify_mask(m):
    """m: [S(q), S(t)] bool. Returns cls[ti][qj] over [t=128, q=512] tiles.

    Verifies that every partial tile matches the causal pattern the
    on-device affine_select applies (keep where t <= q).
    """
    cls = np.zeros((NT, NQB), dtype=np.int64)
    for ti in range(NT):
        t0 = ti * PT
        for qj in range(NQB):
            q0 = qj * NQ
            sub = m[q0 : q0 + NQ, t0 : t0 + PT]  # [q, t]
            if sub.all():
                cls[ti][qj] = FULL
            elif not sub.any():
                cls[ti][qj] = SKIP
            else:
                tt, qq = np.meshgrid(np.arange(PT), np.arange(NQ))
                causal = (t0 + tt) <= (q0 + qq)  # [q, t]
                if not np.array_equal(sub, causal):
                    raise NotImplementedError(
                        "only causal or all-true masks are supported"
                    )
                cls[ti][qj] = PARTIAL
    # every query row must attend to at least one key (else Z=0)
    if not m.any(axis=1).all():
        raise NotImplementedError("mask has fully-masked query rows")
    return cls


_PROGRAM_CACHE = {}


def _get_program(mask, use_bias):
    key = (mask.tobytes(), use_bias)
    prog = _PROGRAM_CACHE.get(key)
    if prog is None:
        cls = _classify_mask(mask)
        prog = build_program(cls, use_bias)
        _PROGRAM_CACHE[key] = prog
    return prog


def _make_in_maps(inputs):
    x = np.asarray(inputs["x"], dtype=np.float32)
    wq = np.asarray(inputs["wq"], dtype=np.float32)
    wk = np.asarray(inputs["wk"], dtype=np.float32)
    wv = np.asarray(inputs["wv"], dtype=np.float32)
    wo = np.asarray(inputs["wo"], dtype=np.float32)
    bq = np.asarray(inputs["bq"], dtype=np.float32)
    bk = np.asarray(inputs["bk"], dtype=np.float32)
    bv = np.asarray(inputs["bv"], dtype=np.float32)
    use_bias = bool(bq.any() or bk.any() or bv.any())

    # xT [KD, PT, S] contiguous per k-tile
    xT = [
        np.ascontiguousarray(
            x[b].T.reshape(KD, PT, S).astype(NPDT)
        )
        for b in range(B)
    ]
    wkv_full = np.concatenate([wk, wv], axis=1)  # [D, 2*DK]
    wkv_r = np.ascontiguousarray(
        wkv_full.reshape(KD, PT, 2 * DK).transpose(1, 0, 2).astype(NPDT)
    )
    in_maps = []
    for c in range(NCORES):
        b, g = divmod(c, GROUPS)
        wq_g = wq[:, g * GD : (g + 1) * GD]
        wq_r = np.ascontiguousarray(
            wq_g.reshape(KD, PT, GD).transpose(1, 0, 2).astype(NPDT)
        )
        wo_g = wo[g * GD : (g + 1) * GD, :]
        wo_r = np.ascontiguousarray(
            wo_g.reshape(GD // PT, PT, D).transpose(1, 0, 2).astype(NPDT)
        )
        im = {"xT": xT[b], "wq": wq_r, "wkv": wkv_r, "wo": wo_r}
        if use_bias:
            im["bq"] = np.ascontiguousarray(
                bq[g * GD : (g + 1) * GD]
            ).reshape(1, GD).astype(NPDT)
            im["bk"] = bk.reshape(1, DK).astype(NPDT)
            im["bv"] = bv.reshape(1, DK).astype(NPDT)
        in_maps.append(im)
    return in_maps


def kernel(x, mask, wq, bq, wk, bk, wv, bv, wo, bo):
    mask = np.asarray(mask).astype(bool).reshape(S, S)
    bq = np.asarray(bq, dtype=np.float32)
    bk = np.asarray(bk, dtype=np.float32)
    bv = np.asarray(bv, dtype=np.float32)
    bo = np.asarray(bo, dtype=np.float32)

    use_bias = bool(bq.any() or bk.any() or bv.any())
    nc = _get_program(mask, use_bias)

    in_maps = _make_in_maps(
        {"x": x, "wq": wq, "wk": wk, "wv": wv, "wo": wo,
         "bq": bq, "bk": bk, "bv": bv}
    )

    res = bass_utils.run_bass_kernel_spmd(nc, in_maps, core_ids=list(range(NCORES)))

    out = np.zeros((B, S, D), dtype=np.float32)
    for c in range(NCORES):
        b = c // GROUPS
        out[b] += np.asarray(res.results[c]["out"], dtype=np.float32)
    out += bo
    return out
